# revision 1
# baseline (speedup 1.0000x reference)
"""MHA kernel for TRN2: B=4,T=2048,D=1024,H=16,HD=64 across 8 NeuronCores.

Sharding: core c -> batch c//2, query half c%2 (host rotates the sequence so
each core's queries are rows 0:1024; softmax over keys is permutation
invariant). No collectives.

Design:
- bf16 everywhere; x^T and W^T are pre-transposed/cast on the host and
  uploaded directly, so there are zero on-device transposes.
- All matmuls stream 512-wide moving operands (1 cycle/row on TensorE); PSUM
  accumulates in fp32. A ones-column folded into V yields the softmax
  denominators from the same PV matmul.
- The exp (ScalarE) is the secondary bottleneck; V for heads 8-15 and the
  Q/K projections of later groups are software-pipelined as PE "filler"
  micro-ops between attention steps so TensorE never waits on the exp.
- Softmax normalization: DVE reciprocal of the denominators, GpSimd
  partition_broadcast, DVE multiply -- off TensorE's critical path.
- Final projection of the first query blocks overlaps the last attention
  group; per-tile DMA out.
TensorE is ~95% occupied end-to-end in the CoreSim cost model (~402 us/core).
"""
import sys
sys.path.insert(0, "/opt/trn_rl_repo")
import warnings
warnings.filterwarnings("ignore")

import numpy as np
import ml_dtypes
import concourse.bass as bass
import concourse.mybir as mybir
import concourse.tile as tile
from concourse import bacc
from concourse.bass_utils import run_bass_kernel_spmd

F32 = mybir.dt.float32
BF = mybir.dt.bfloat16
EXP = mybir.ActivationFunctionType.Exp
MUL = mybir.AluOpType.mult

T, D = 2048, 1024
TQ = 1024          # queries per core
NG = 8             # head groups (2 heads each)
NSC = 16           # s chunks of 128
NDC = 8            # d chunks of 128
SCALE = 0.125      # 1/sqrt(64)


def build_nc():
    nc = bacc.Bacc("TRN2", target_bir_lowering=False, debug=False, num_devices=8)
    xt = nc.dram_tensor("xt", [NDC, 128, T], BF, kind="ExternalInput")
    wqt = nc.dram_tensor("wqt", [NDC, 128, D], BF, kind="ExternalInput")
    wkt = nc.dram_tensor("wkt", [NDC, 128, D], BF, kind="ExternalInput")
    wvt = nc.dram_tensor("wvt", [NDC, 128, D], BF, kind="ExternalInput")
    wot = nc.dram_tensor("wot", [NDC, 128, D], BF, kind="ExternalInput")
    bo = nc.dram_tensor("bo", [1, D], F32, kind="ExternalInput")
    y = nc.dram_tensor("y", [TQ, D], F32, kind="ExternalOutput")

    with tile.TileContext(nc) as tc:
        with (
            tc.tile_pool(name="persist", bufs=1) as pp,
            tc.tile_pool(name="qk", bufs=3) as qk,
            tc.tile_pool(name="ptp", bufs=3) as ptp,
            tc.tile_pool(name="small", bufs=2) as sp,
            tc.tile_pool(name="yp", bufs=2) as yp,
            tc.tile_pool(name="ps_work", bufs=2, space="PSUM") as psw,
            tc.tile_pool(name="ps_pv", bufs=1, space="PSUM") as psv,
            tc.tile_pool(name="ps_log", bufs=2, space="PSUM") as psl,
        ):
            bias = pp.tile([128, D], F32)

            # first V matmul needs wv cols 0:512 + x cols 0:128; interleave so
            # PE can start ~5us earlier (bias DMA deferred: only read by the
            # out-projection, keep it off the first dispatch slots)
            wvTs = pp.tile([128, NDC, D], BF)
            xTs = pp.tile([128, NDC, T], BF)
            for dc in range(NDC):
                nc.sync.dma_start(out=wvTs[:, dc, 0:512], in_=wvt[dc][:, 0:512])
                nc.sync.dma_start(out=xTs[:, dc, 0:512], in_=xt[dc][:, 0:512])
            nc.sync.dma_start(
                out=bias, in_=bass.AP(tensor=bo, offset=0, ap=[[0, 128], [1, D]]))
            for tq in range(1, 4):
                ts = slice(tq * 512, (tq + 1) * 512)
                for dc in range(NDC):
                    nc.sync.dma_start(out=xTs[:, dc, ts], in_=xt[dc][:, ts])
            wqTs = pp.tile([128, NDC, D], BF)
            wkTs = pp.tile([128, NDC, D], BF)
            for dc in range(NDC):
                nc.sync.dma_start(out=wqTs[:, dc, :], in_=wqt[dc])
                nc.sync.dma_start(out=wkTs[:, dc, :], in_=wkt[dc])
            for dc in range(NDC):
                nc.sync.dma_start(
                    out=wvTs[:, dc, 512:1024], in_=wvt[dc][:, 512:1024])
            woTs = pp.tile([128, NDC, D], BF)
            for dc in range(NDC):
                nc.sync.dma_start(out=woTs[:, dc, :], in_=wot[dc])

            catT = [pp.tile([128, TQ], BF, name=f"catT{g}") for g in range(NG)]
            vt = pp.tile([128, NSC, 16, 65], BF)

            # ---- V as micro-ops: vt[s, sc, h, 0:64], col 64 = ones ----
            def v_ops(sc, hh):
                ops = []
                p = psw.tile([128, 512], F32, tag="work")
                for dc in range(NDC):
                    def mm(p=p, dc=dc, sc=sc, hh=hh):
                        nc.tensor.matmul(
                            p, xTs[:, dc, sc * 128:(sc + 1) * 128],
                            wvTs[:, dc, hh * 512:(hh + 1) * 512],
                            start=(dc == 0), stop=(dc == NDC - 1))
                    ops.append(mm)
                def cp(p=p, sc=sc, hh=hh):
                    nc.vector.tensor_copy(
                        out=vt[:, sc, hh * 8:(hh + 1) * 8, 0:64],
                        in_=p.rearrange("p (h c) -> p h c", h=8))
                ops.append(cp)
                return ops

            nc.vector.memset(vt[:, :, :, 64:65], 1.0)
            # heads 0-7 (groups 0-3) upfront; heads 8-15 deferred to fillers
            # except 4 chunks emitted inline to relieve filler pop capacity
            for sc in range(NSC):
                for op in v_ops(sc, 0):
                    op()

            # ---- projection micro-op queue (PE fillers for attention) ----
            proj_tiles = {}

            def start_proj(g):
                qTg = qk.tile([128, TQ], BF, tag="qT", name=f"qT{g}")
                kTg = qk.tile([128, T], BF, tag="kT", name=f"kT{g}")
                proj_tiles[g] = (qTg, kTg)
                ops = []
                for dst, blk, ws in ((qTg, 0, wqTs), (kTg, 0, wkTs),
                                     (kTg, 1, wkTs), (kTg, 2, wkTs),
                                     (kTg, 3, wkTs), (qTg, 1, wqTs)):
                    if True:
                        p = psw.tile([128, 512], F32, tag="work")
                        for dc in range(NDC):
                            def mm(p=p, dc=dc, blk=blk, ws=ws, g=g):
                                nc.tensor.matmul(
                                    p, ws[:, dc, g * 128:(g + 1) * 128],
                                    xTs[:, dc, blk * 512:(blk + 1) * 512],
                                    start=(dc == 0), stop=(dc == NDC - 1))
                            ops.append(mm)
                        def cp(p=p, dst=dst, blk=blk):
                            nc.vector.tensor_copy(
                                out=dst[:, blk * 512:(blk + 1) * 512], in_=p)
                        ops.append(cp)
                return ops

            filler = []
            queued = [0]
            popped = [0]
            need_before = {}

            def queue_filler(ops):
                filler.extend(ops)
                queued[0] += len(ops)

            def pop_filler(n):
                for _ in range(n):
                    if filler:
                        filler.pop(0)()
                        popped[0] += 1

            def drain_until(k):
                while popped[0] < k and filler:
                    filler.pop(0)()
                    popped[0] += 1

            # upfront: projections for groups 0 and 1
            for op in start_proj(0):
                op()
            for op in start_proj(1):
                op()
            # deferred V for heads 8-15: 4 chunks inline (free ACT-idle head
            # room), a sliver queued now, the bulk after the earlier-deadline
            # proj(2)/proj(3) ops (deadline-ordered FIFO, no drain bursts)
            for sc in range(6):
                queue_filler(v_ops(sc, 1))
            need_before[(4, 0)] = queued[0]

            # ---- final-projection micro-ops (tail overlap fillers) ----
            def outproj_ops(qb):
                ops = []
                yt = yp.tile([128, D], F32, tag="yt", name=f"yt{qb}")
                for nh in range(2):
                    p = psw.tile([128, 512], F32, tag="work", name=f"op{qb}{nh}")
                    for g in range(NG):
                        def mm(p=p, g=g, nh=nh, qb=qb):
                            nc.tensor.matmul(
                                p, catT[g][:, qb * 128:(qb + 1) * 128],
                                woTs[:, g, nh * 512:(nh + 1) * 512],
                                start=(g == 0), stop=(g == NG - 1))
                        ops.append(mm)
                    def add(p=p, yt=yt, nh=nh):
                        nc.vector.tensor_add(
                            out=yt[:, nh * 512:(nh + 1) * 512], in0=p,
                            in1=bias[:, nh * 512:(nh + 1) * 512])
                    ops.append(add)
                    def dma(yt=yt, qb=qb, nh=nh):
                        nc.sync.dma_start(
                            out=y[qb * 128:(qb + 1) * 128,
                                  nh * 512:(nh + 1) * 512],
                            in_=yt[:, nh * 512:(nh + 1) * 512])
                    ops.append(dma)
                return ops

            # ---- attention: flattened (g, qh, sc) pipeline with the PV
            # matmuls delayed one step behind logits/exp, so PE crosses
            # (g, qh) boundaries without waiting on the exp backlog ----
            pv_tiles = {}
            prev = None  # (g, qh, sc, pt)

            def emit_pv(g, qh, sc, pt):
                if (g, qh) not in pv_tiles:
                    pv_tiles[(g, qh)] = psv.tile(
                        [65, 2, 512], F32, tag="pv", name=f"pv{g}_{qh}")
                pv = pv_tiles[(g, qh)]
                nc.tensor.matmul(
                    pv[:, 0, :], vt[:, sc, 2 * g, :], pt[:, 0, :],
                    start=(sc == 0), stop=(sc == NSC - 1))
                nc.tensor.matmul(
                    pv[:, 1, :], vt[:, sc, 2 * g + 1, :], pt[:, 1, :],
                    start=(sc == 0), stop=(sc == NSC - 1))
                if sc == NSC - 1:
                    # free the psum banks fast, then normalize on Pool
                    qs = slice(qh * 512, (qh + 1) * 512)
                    pvs = sp.tile([65, 2, 512], F32, tag="pvs")
                    nc.vector.tensor_copy(out=pvs, in_=pv)
                    del pv_tiles[(g, qh)]
                    for h in range(2):
                        rec = sp.tile([1, 512], F32, tag="rec")
                        nc.vector.reciprocal(out=rec, in_=pvs[64:65, h, :])
                        zb = sp.tile([64, 512], F32, tag="zb")
                        nc.gpsimd.partition_broadcast(zb, rec)
                        nc.vector.tensor_tensor(
                            out=catT[g][h * 64:(h + 1) * 64, qs],
                            in0=pvs[0:64, h, :], in1=zb, op=MUL)

            for g in range(NG):
                if g + 2 < NG:
                    queue_filler(start_proj(g + 2))
                    need_before[(g + 2, 0)] = queued[0] - 27
                    need_before[(g + 2, 1)] = queued[0]
                if g == 1:
                    for sc in range(6, NSC):
                        queue_filler(v_ops(sc, 1))
                    need_before[(4, 0)] = queued[0]
                qTg, kTg = proj_tiles[g]
                for qh in range(2):
                    drain_until(need_before.get((g, qh), 0))
                    if g == NG - 1 and qh == 1:
                        # catT[7] cols 0:512 are done; overlap out-proj of the
                        # first query blocks with the last attention pass
                        for qb in range(2):
                            queue_filler(outproj_ops(qb))
                    qs = slice(qh * 512, (qh + 1) * 512)
                    for sc in range(NSC):
                        lg = psl.tile([128, 2, 512], F32, tag="lg")
                        nc.tensor.matmul(
                            lg[:, 0, :], kTg[0:64, sc * 128:(sc + 1) * 128],
                            qTg[0:64, qs], start=True, stop=True)
                        nc.tensor.matmul(
                            lg[:, 1, :], kTg[64:128, sc * 128:(sc + 1) * 128],
                            qTg[64:128, qs], start=True, stop=True)
                        pt = ptp.tile([128, 2, 512], BF, tag="pt")
                        nc.scalar.activation(
                            out=pt.rearrange("p a b -> p (a b)"),
                            in_=lg.rearrange("p a b -> p (a b)"),
                            func=EXP, scale=SCALE)
                        if prev is not None:
                            emit_pv(*prev)
                        prev = (g, qh, sc, pt)
                        pop_filler(2)
            emit_pv(*prev)
            pop_filler(len(filler))

            # ---- final projection (remaining query blocks) ----
            for qb in range(2, 7):
                for op in outproj_ops(qb):
                    op()
            yt7 = yp.tile([128, D], F32, tag="yt")
            for nh in range(2):
                p = psw.tile([128, 512], F32, tag="work")
                for g in range(NG):
                    nc.tensor.matmul(
                        p, catT[g][:, 896:1024],
                        woTs[:, g, nh * 512:(nh + 1) * 512],
                        start=(g == 0), stop=(g == NG - 1))
                for q4 in range(2):
                    cs = slice(nh * 512 + q4 * 256, nh * 512 + (q4 + 1) * 256)
                    ps = slice(q4 * 256, (q4 + 1) * 256)
                    nc.vector.tensor_add(
                        out=yt7[:, cs], in0=p[:, ps], in1=bias[:, cs])
                    nc.sync.dma_start(out=y[896:1024, cs], in_=yt7[:, cs])

    nc.compile()
    return nc


def make_in_maps(x, wq2, wk2, wv2, wo2, bo2):
    """Per-core input dicts from full (already 2-D) fp32 arrays."""
    bf = ml_dtypes.bfloat16
    wqt = np.ascontiguousarray(wq2.T.astype(bf).reshape(NDC, 128, D))
    wkt = np.ascontiguousarray(wk2.T.astype(bf).reshape(NDC, 128, D))
    wvt = np.ascontiguousarray(wv2.T.astype(bf).reshape(NDC, 128, D))
    wot = np.ascontiguousarray(wo2.T.astype(bf).reshape(NDC, 128, D))
    bo3 = np.ascontiguousarray(bo2.reshape(1, D), dtype=np.float32)
    in_maps = []
    for c in range(8):
        b, h = c // 2, c % 2
        xr = x[b] if h == 0 else np.concatenate([x[b, TQ:], x[b, :TQ]], axis=0)
        xtc = np.ascontiguousarray(xr.T.astype(bf).reshape(NDC, 128, T))
        in_maps.append({"xt": xtc, "wqt": wqt, "wkt": wkt, "wvt": wvt,
                        "wot": wot, "bo": bo3})
    return in_maps


_CACHE = {}


def kernel(x, Wq, Wk, Wv, Wo, bo):
    if "nc" not in _CACHE:
        _CACHE["nc"] = build_nc()
    nc = _CACHE["nc"]
    x = np.ascontiguousarray(x, dtype=np.float32)
    in_maps = make_in_maps(
        x, np.asarray(Wq).reshape(D, D).astype(np.float32),
        np.asarray(Wk).reshape(D, D).astype(np.float32),
        np.asarray(Wv).reshape(D, D).astype(np.float32),
        np.asarray(Wo).astype(np.float32),
        np.asarray(bo).astype(np.float32))
    res = run_bass_kernel_spmd(nc, in_maps, core_ids=list(range(8)))
    out = np.empty((4, T, D), dtype=np.float32)
    for c in range(8):
        b, h = c // 2, c % 2
        out[b, h * TQ:(h + 1) * TQ] = res.results[c]["y"]
    return out

